# revision 2
# baseline (speedup 1.0000x reference)
"""Trainium2 Bass kernel for nn_Attention_C_12111807775306.

Structure exploited: in the reference, attention_ca's output feeds ONLY the
top-k expert selection (computed from batch element 0 alone); the expert conv
branches apply to the original input I. So the first channel-attention runs on
the host (cheap numpy, batch 0 only) to pick the 4 experts, and the device
kernel computes: 4 expert convs on I -> concat -> 3x3 conv (768->192) ->
kv/q convs -> channel attention -> 1x1 proj.

Sharding: 4 cores, one batch element per core (cores 0-3 of 8).
Compute dtype bf16 (fp32 PSUM accumulation), fp32 output.
"""
import sys
sys.path.insert(0, "/opt/trn_rl_repo")
import numpy as np
import ml_dtypes

DIM = 192
HEADS = 6
B = 4
H = 64
W = 64
L = H * W
TOPK = 4
PADS = [0, 1, 2] * 4
KSZ = [1, 3, 5] * 4
GROUPS = [1] * 6 + [DIM] * 6
PAD = 4          # host-side zero padding of I on each border
S = 72           # padded spatial size
BF16 = ml_dtypes.bfloat16


def _l2n(x):
    return x / np.maximum(np.linalg.norm(x, axis=-1, keepdims=True), 1e-12)


def _select_experts(I, T, ca1_proj_w):
    """Replicate attention_ca + binning for batch 0 only; return top-4 idx."""
    b0I = I[0].astype(np.float64)
    b0T = T[0].astype(np.float64)
    pooled = b0T.reshape(DIM // 4, 4, L).mean(1)          # [48, L]
    q = _l2n(b0I.reshape(HEADS, DIM // HEADS, L))
    k = _l2n(pooled.reshape(HEADS, 8, L))
    kt = np.tile(k, (1, 4, 1))
    s = np.einsum("hcl,hdl->hcd", q, kt)
    s = s - s.max(-1, keepdims=True)
    e = np.exp(s)
    attn = e / e.sum(-1, keepdims=True)
    out = np.einsum("hcd,hdl->hcl", attn, kt).reshape(DIM, H, W)
    fmap0 = np.einsum("oi,ihw->ohw", ca1_proj_w[:, :, 0, 0].astype(np.float64), out)
    m = fmap0.mean(axis=(0, 1))                            # [W]
    bins = np.array([m[(i * W) // 12: -(-((i + 1) * W) // 12)].mean()
                     for i in range(12)])
    # softmax is monotone; jax.lax.top_k breaks ties toward lower index,
    # matching stable argsort on the negated values.
    return list(np.argsort(-bins, kind="stable")[:TOPK])


def _build_and_run(sel, host_inputs):
    import concourse.mybir as mybir
    import concourse.tile as tile
    from concourse import bacc
    from concourse.bass_utils import run_bass_kernel_spmd

    bf = mybir.dt.bfloat16
    f32 = mybir.dt.float32

    nc = bacc.Bacc("TRN2", target_bir_lowering=False, debug=False,
                   enable_asserts=False, num_devices=B)

    # ---- DRAM I/O declarations (shapes mirror host prep below) ----
    xin_d = nc.dram_tensor("xin", [DIM, S, S], bf, kind="ExternalInput")
    dense_d = {}
    diag_d = {}
    for i, j in enumerate(sel):
        kk = KSZ[j] * KSZ[j]
        if GROUPS[j] == 1:
            dense_d[i] = nc.dram_tensor(f"e{i}_w", [DIM, kk * DIM], bf,
                                        kind="ExternalInput")
        elif KSZ[j] > 1:
            diag_d[i] = (
                nc.dram_tensor(f"e{i}_da", [128, kk * 128], bf, kind="ExternalInput"),
                nc.dram_tensor(f"e{i}_db", [64, kk * 64], bf, kind="ExternalInput"),
            )
    wexout_d = nc.dram_tensor("wexout", [DIM, TOPK * 9 * DIM], bf,
                              kind="ExternalInput")
    dkv_d = [nc.dram_tensor(f"dkv{m}", [128, 9 * 128], bf, kind="ExternalInput")
             for m in range(3)]
    dq_d = (nc.dram_tensor("dq_a", [128, 9 * 128], bf, kind="ExternalInput"),
            nc.dram_tensor("dq_b", [64, 9 * 64], bf, kind="ExternalInput"))
    kvw_d = nc.dram_tensor("kvw", [DIM, 2 * DIM], bf, kind="ExternalInput")
    projw_d = nc.dram_tensor("projw", [DIM, DIM], bf, kind="ExternalInput")
    ident_d = nc.dram_tensor("ident", [128, 128], bf, kind="ExternalInput")
    # per-channel f32 vectors: cols 0..3 = bias of expert i (0 if folded into
    # ACT path), 4 = ex_out bias, 5 = w9-style dw1x1 scale (unused cols zero),
    # 6 = temperature per channel
    vecs_d = nc.dram_tensor("vecs", [DIM, 8], f32, kind="ExternalInput")
    out_d = nc.dram_tensor("out", [DIM, L], f32, kind="ExternalOutput")

    CH = [(0, 128), (128, 64)]   # (start, size) channel chunks for 192

    with tile.TileContext(nc) as tc:
        with tc.tile_pool(name="persist", bufs=1) as pp, \
             tc.tile_pool(name="psA", bufs=2, space="PSUM") as psA, \
             tc.tile_pool(name="psB", bufs=2, space="PSUM") as psB, \
             tc.tile_pool(name="psS", bufs=4, space="PSUM") as psS, \
             tc.tile_pool(name="dram", bufs=1, space="DRAM") as dramp:

            vecs = [pp.tile([128, 8], f32, tag="vec_a", name="vec_a"),
                    pp.tile([64, 8], f32, tag="vec_b", name="vec_b")]
            nc.sync.dma_start(vecs[0][:], vecs_d.ap()[0:128, :])
            nc.sync.dma_start(vecs[1][:], vecs_d.ap()[128:192, :])
            ident = pp.tile([128, 128], bf, tag="ident", name="ident")
            nc.sync.dma_start(ident[:], ident_d.ap()[:, :])
            kvw = [pp.tile([128, 2 * DIM], bf, tag="kvw_a", name="kvw_a"),
                   pp.tile([64, 2 * DIM], bf, tag="kvw_b", name="kvw_b")]
            nc.sync.dma_start(kvw[0][:], kvw_d.ap()[0:128, :])
            nc.sync.dma_start(kvw[1][:], kvw_d.ap()[128:192, :])
            projw = [pp.tile([128, DIM], bf, tag="pw_a", name="pw_a"),
                     pp.tile([64, DIM], bf, tag="pw_b", name="pw_b")]
            nc.sync.dma_start(projw[0][:], projw_d.ap()[0:128, :])
            nc.sync.dma_start(projw[1][:], projw_d.ap()[128:192, :])
            dq = [pp.tile([128, 9, 128], bf, tag="dq_a", name="dq_a"),
                  pp.tile([64, 9, 64], bf, tag="dq_b", name="dq_b")]
            nc.sync.dma_start(dq[0][:], dq_d[0].ap().rearrange("p (s c) -> p s c", s=9))
            nc.sync.dma_start(dq[1][:], dq_d[1].ap().rearrange("p (s c) -> p s c", s=9))
            dkv = [pp.tile([128, 9, 128], bf, tag=f"dkv{m}", name=f"dkv{m}") for m in range(3)]
            for m in range(3):
                nc.sync.dma_start(dkv[m][:],
                                  dkv_d[m].ap().rearrange("p (s c) -> p s c", s=9))

            # fmap2 lives across phase 1 -> 2
            fmap2 = [pp.tile([128, S, S], bf, tag="fm_a", name="fm_a"),
                     pp.tile([64, S, S], bf, tag="fm_b", name="fm_b")]
            nc.gpsimd.memset(fmap2[0][:], 0.0)
            nc.gpsimd.memset(fmap2[1][:], 0.0)

            # ---------------- Phase 1: experts + ex_out ----------------
            with tc.tile_pool(name="ph1", bufs=1) as p1:
                xin = [p1.tile([128, S, S], bf, tag="x_a", name="x_a"),
                       p1.tile([64, S, S], bf, tag="x_b", name="x_b")]
                nc.sync.dma_start(
                    xin[0][:], xin_d.ap()[0:128, :, :])
                nc.sync.dma_start(
                    xin[1][:], xin_d.ap()[128:192, :, :])

                dense_w = {}
                diag_w = {}
                for i, j in enumerate(sel):
                    kk = KSZ[j] * KSZ[j]
                    if GROUPS[j] == 1:
                        dense_w[i] = [
                            p1.tile([128, kk, DIM], bf, tag=f"dw{i}_a", name=f"dw{i}_a"),
                            p1.tile([64, kk, DIM], bf, tag=f"dw{i}_b", name=f"dw{i}_b")]
                        nc.sync.dma_start(
                            dense_w[i][0][:],
                            dense_d[i].ap()[0:128, :].rearrange(
                                "p (s c) -> p s c", s=kk))
                        nc.sync.dma_start(
                            dense_w[i][1][:],
                            dense_d[i].ap()[128:192, :].rearrange(
                                "p (s c) -> p s c", s=kk))
                    elif KSZ[j] > 1:
                        diag_w[i] = [
                            p1.tile([128, kk, 128], bf, tag=f"gw{i}_a", name=f"gw{i}_a"),
                            p1.tile([64, kk, 64], bf, tag=f"gw{i}_b", name=f"gw{i}_b")]
                        nc.sync.dma_start(
                            diag_w[i][0][:],
                            diag_d[i][0].ap().rearrange("p (s c) -> p s c", s=kk))
                        nc.sync.dma_start(
                            diag_w[i][1][:],
                            diag_d[i][1].ap().rearrange("p (s c) -> p s c", s=kk))
                wexout = [p1.tile([128, TOPK, 9, DIM], bf, tag="wx_a", name="wx_a"),
                          p1.tile([64, TOPK, 9, DIM], bf, tag="wx_b", name="wx_b")]
                nc.sync.dma_start(
                    wexout[0][:], wexout_d.ap()[0:128, :].rearrange(
                        "p (e s c) -> p e s c", e=TOPK, s=9))
                nc.sync.dma_start(
                    wexout[1][:], wexout_d.ap()[128:192, :].rearrange(
                        "p (e s c) -> p e s c", e=TOPK, s=9))

                outs = []
                for i, j in enumerate(sel):
                    ot = [p1.tile([128, S, S], bf, tag=f"o{i}_a", name=f"o{i}_a"),
                          p1.tile([64, S, S], bf, tag=f"o{i}_b", name=f"o{i}_b")]
                    nc.gpsimd.memset(ot[0][:], 0.0)
                    nc.gpsimd.memset(ot[1][:], 0.0)
                    outs.append(ot)
                    ks = KSZ[j]
                    p = PADS[j]
                    shifts = [(dy, dx) for dy in range(ks) for dx in range(ks)]
                    if GROUPS[j] == 1:
                        # dense conv: out[m] = sum_s sum_k W[k, s, m]^T x_s
                        for mi, (m0, msz) in enumerate(CH):
                            for t in range(8):
                                ps = psA.tile([128, 512], f32, tag="big", name="big")
                                nmm = len(shifts) * 2
                                c = 0
                                for si, (dy, dx) in enumerate(shifts):
                                    r0 = PAD + 8 * t + dy - p
                                    c0 = PAD + dx - p
                                    for ki in range(2):
                                        nc.tensor.matmul(
                                            ps[:msz, :],
                                            dense_w[i][ki][:, si, m0:m0 + msz],
                                            xin[ki][:, r0:r0 + 8, c0:c0 + 64],
                                            start=(c == 0), stop=(c == nmm - 1))
                                        c += 1
                                nc.vector.tensor_scalar_add(
                                    ot[mi][:, PAD + 8 * t:PAD + 8 * t + 8,
                                           PAD:PAD + 64],
                                    ps[:msz, :].rearrange("p (r c) -> p r c", r=8),
                                    vecs[mi][:, i:i + 1])
                    elif ks > 1:
                        # depthwise via diagonal matmuls
                        for mi, (m0, msz) in enumerate(CH):
                            for t in range(8):
                                ps = psA.tile([128, 512], f32, tag="big", name="big")
                                for si, (dy, dx) in enumerate(shifts):
                                    r0 = PAD + 8 * t + dy - p
                                    c0 = PAD + dx - p
                                    nc.tensor.matmul(
                                        ps[:msz, :],
                                        diag_w[i][mi][:, si, :],
                                        xin[mi][:, r0:r0 + 8, c0:c0 + 64],
                                        start=(si == 0),
                                        stop=(si == len(shifts) - 1))
                                nc.vector.tensor_scalar_add(
                                    ot[mi][:, PAD + 8 * t:PAD + 8 * t + 8,
                                           PAD:PAD + 64],
                                    ps[:msz, :].rearrange("p (r c) -> p r c", r=8),
                                    vecs[mi][:, i:i + 1])
                    else:
                        # depthwise 1x1: per-channel scale + bias on ACT
                        scol = 5 if [x for x in sel[:i]
                                     if GROUPS[x] == DIM and KSZ[x] == 1] == [] else 7
                        for mi, (m0, msz) in enumerate(CH):
                            nc.vector.tensor_scalar(
                                ot[mi][:, PAD:PAD + 64, PAD:PAD + 64],
                                xin[mi][:, PAD:PAD + 64, PAD:PAD + 64],
                                vecs[mi][:, scol:scol + 1],
                                vecs[mi][:, i:i + 1],
                                op0=mybir.AluOpType.mult,
                                op1=mybir.AluOpType.add)

                # ex_out: fmap2 = sum_e conv3x3(outs_e, block_e) + bias
                for mi, (m0, msz) in enumerate(CH):
                    for t in range(8):
                        ps = psA.tile([128, 512], f32, tag="big", name="big")
                        nmm = TOPK * 9 * 2
                        c = 0
                        for e in range(TOPK):
                            for si in range(9):
                                dy, dx = si // 3, si % 3
                                r0 = PAD + 8 * t + dy - 1
                                c0 = PAD + dx - 1
                                for ki in range(2):
                                    nc.tensor.matmul(
                                        ps[:msz, :],
                                        wexout[ki][:, e, si, m0:m0 + msz],
                                        outs[e][ki][:, r0:r0 + 8, c0:c0 + 64],
                                        start=(c == 0), stop=(c == nmm - 1))
                                    c += 1
                        nc.vector.tensor_scalar_add(
                            fmap2[mi][:, PAD + 8 * t:PAD + 8 * t + 8,
                                      PAD:PAD + 64],
                            ps[:msz, :].rearrange("p (r c) -> p r c", r=8),
                            vecs[mi][:, 4:5])

            # ---------------- Phase 2: attention ----------------
            with tc.tile_pool(name="ph2", bufs=1) as p2:
                # kv_pre = 1x1(fmap2) over region [3:69)^2
                kvpre = [p2.tile([128, S, S], bf, tag=f"kp{m}", name=f"kp{m}") for m in range(3)]
                row_tiles = [(3 + 7 * t, 7) for t in range(9)] + [(66, 3)]
                for m in range(3):
                    for (r0, rc) in row_tiles:
                        ps = psB.tile([128, 512], f32, tag="kvp", name="kvp")
                        for ki in range(2):
                            nc.tensor.matmul(
                                ps[:, :rc * 66],
                                kvw[ki][:, 128 * m:128 * (m + 1)],
                                fmap2[ki][:, r0:r0 + rc, 3:69],
                                start=(ki == 0), stop=(ki == 1))
                        nc.vector.tensor_copy(
                            kvpre[m][:, r0:r0 + rc, 3:69],
                            ps[:, :rc * 66].rearrange("p (r c) -> p r c", r=rc))

                # q = dw3x3(fmap2); k,v = dw3x3(kv_pre)
                q_sb = [p2.tile([128, L], bf, tag="q_a", name="q_a"),
                        p2.tile([64, L], bf, tag="q_b", name="q_b")]
                k_sb = [p2.tile([128, L], bf, tag="k_a", name="k_a"),
                        p2.tile([64, L], bf, tag="k_b", name="k_b")]
                v_sb = [p2.tile([32, L], bf, tag=f"v{h}", name=f"v{h}")
                        for h in range(HEADS)]

                def dw3x3(writes, dst_cols, diag, src):
                    for t in range(8):
                        ps = psB.tile([128, 512], f32, tag="kvp", name="kvp")
                        for si in range(9):
                            dy, dx = si // 3, si % 3
                            r0 = PAD + 8 * t + dy - 1
                            c0 = PAD + dx - 1
                            nc.tensor.matmul(
                                ps[:dst_cols, :],
                                diag[:, si, :],
                                src[:, r0:r0 + 8, c0:c0 + 64],
                                start=(si == 0), stop=(si == 8))
                        for (dst, p0, sz) in writes:
                            nc.vector.tensor_copy(
                                dst[:, 512 * t:512 * (t + 1)],
                                ps[p0:p0 + sz, :])

                dw3x3([(q_sb[0], 0, 128)], 128, dq[0], fmap2[0])
                dw3x3([(q_sb[1], 0, 64)], 64, dq[1], fmap2[1])
                # k: kv channels 0..191
                dw3x3([(k_sb[0], 0, 128)], 128, dkv[0], kvpre[0])
                dw3x3([(k_sb[1], 0, 64)], 64, dkv[1][0:64, :, 0:64],
                      kvpre[1][0:64])
                # v: kv channels 192..383 -> per-head tiles
                dw3x3([(v_sb[0], 0, 32), (v_sb[1], 32, 32)], 64,
                      dkv[1][64:128, :, 64:128], kvpre[1][64:128])
                dw3x3([(v_sb[2 + i], 32 * i, 32) for i in range(4)], 128,
                      dkv[2], kvpre[2])

                # sum of squares for q,k norms
                sq = p2.tile([128, L], bf, tag="sq", name="sq")
                qss = [p2.tile([128, 1], f32, tag="qss_a", name="qss_a"),
                       p2.tile([64, 1], f32, tag="qss_b", name="qss_b")]
                kss = [p2.tile([128, 1], f32, tag="kss_a", name="kss_a"),
                       p2.tile([64, 1], f32, tag="kss_b", name="kss_b")]
                for src, dst in ((q_sb, qss), (k_sb, kss)):
                    for ci in range(2):
                        n = 128 if ci == 0 else 64
                        nc.vector.tensor_mul(sq[:n, :], src[ci][:], src[ci][:])
                        nc.vector.reduce_sum(dst[ci][:], sq[:n, :],
                                             axis=mybir.AxisListType.X)
                # inv-norm with temperature folded into q scale
                qsc = [p2.tile([128, 1], f32, tag="qsc_a", name="qsc_a"),
                       p2.tile([64, 1], f32, tag="qsc_b", name="qsc_b")]
                ksc = [p2.tile([128, 1], f32, tag="ksc_a", name="ksc_a"),
                       p2.tile([64, 1], f32, tag="ksc_b", name="ksc_b")]
                for ci in range(2):
                    n = 128 if ci == 0 else 64
                    nc.scalar.activation(qsc[ci][:], qss[ci][:],
                                         mybir.ActivationFunctionType.Sqrt)
                    nc.vector.reciprocal(qsc[ci][:], qsc[ci][:])
                    nc.vector.tensor_mul(qsc[ci][:], qsc[ci][:],
                                         vecs[ci][:, 6:7])
                    nc.scalar.activation(ksc[ci][:], kss[ci][:],
                                         mybir.ActivationFunctionType.Sqrt)
                    nc.vector.reciprocal(ksc[ci][:], ksc[ci][:])
                    # normalize k rows in place (folds the k inv-norm so the
                    # Gram needs no free-dim scale afterwards)
                    nc.vector.tensor_scalar_mul(k_sb[ci][:], k_sb[ci][:],
                                                ksc[ci][:])

                # transpose q,k to [l, c] layout
                qT = p2.tile([128, 32, DIM], bf, tag="qT", name="qT")
                kT = p2.tile([128, 32, DIM], bf, tag="kT", name="kT")
                for src, dst in ((q_sb, qT), (k_sb, kT)):
                    for t in range(32):
                        pt = psS.tile([128, 128], bf, tag="ps_s", name="ps_s")
                        nc.tensor.transpose(
                            pt[:, 0:128], src[0][:, 128 * t:128 * (t + 1)],
                            ident[:])
                        nc.vector.tensor_copy(dst[:, t, 0:128], pt[:, 0:128])
                        pt2 = psS.tile([128, 128], bf, tag="ps_s", name="ps_s")
                        nc.tensor.transpose(
                            pt2[:, 0:64], src[1][:, 128 * t:128 * (t + 1)],
                            ident[0:64, 0:64])
                        nc.vector.tensor_copy(dst[:, t, 128:192], pt2[:, 0:64])

                # per-head attention
                o_sb = [p2.tile([128, L], bf, tag="osb_a", name="osb_a"),
                        p2.tile([64, L], bf, tag="osb_b", name="osb_b")]
                for h in range(HEADS):
                    ci, hb = (0, h) if h < 4 else (1, h - 4)
                    c0 = 32 * hb
                    psg = psS.tile([32, 32], f32, tag="ps_s", name="ps_s")
                    for t in range(32):
                        nc.tensor.matmul(
                            psg[:, :],
                            qT[:, t, 32 * h:32 * h + 32],
                            kT[:, t, 32 * h:32 * h + 32],
                            start=(t == 0), stop=(t == 31))
                    s_sb = p2.tile([32, 32], f32, tag="s_sb", name="s_sb")
                    nc.vector.tensor_scalar_mul(
                        s_sb[:], psg[:], qsc[ci][c0:c0 + 32, :])
                    nmax = p2.tile([32, 1], f32, tag="nmax", name="nmax")
                    nc.vector.reduce_max(nmax[:], s_sb[:],
                                         axis=mybir.AxisListType.X, negate=True)
                    esb = p2.tile([32, 32], f32, tag="esb", name="esb")
                    nc.scalar.activation(esb[:], s_sb[:],
                                         mybir.ActivationFunctionType.Exp,
                                         bias=nmax[:])
                    ssum = p2.tile([32, 1], f32, tag="ssum", name="ssum")
                    nc.vector.reduce_sum(ssum[:], esb[:],
                                         axis=mybir.AxisListType.X)
                    sinv = p2.tile([32, 1], f32, tag="sinv", name="sinv")
                    nc.vector.reciprocal(sinv[:], ssum[:])
                    aT = p2.tile([32, 32], f32, tag="aT", name="aT")
                    nc.vector.transpose(aT[:], esb[:])
                    aTb = p2.tile([32, 32], bf, tag="aTb", name="aTb")
                    nc.vector.tensor_copy(aTb[:], aT[:])
                    for t in range(8):
                        po = psS.tile([32, 512], f32, tag="ps_s", name="ps_s")
                        nc.tensor.matmul(
                            po[:, :], aTb[:],
                            v_sb[h][:, 512 * t:512 * (t + 1)],
                            start=True, stop=True)
                        nc.vector.tensor_scalar_mul(
                            o_sb[ci][c0:c0 + 32, 512 * t:512 * (t + 1)],
                            po[:, :], sinv[:])

                # final 1x1 projection -> fp32 output (streamed)
                for mi, (m0, msz) in enumerate(CH):
                    for t in range(8):
                        ps = psB.tile([128, 512], f32, tag="kvp", name="kvp")
                        for ki in range(2):
                            nc.tensor.matmul(
                                ps[:msz, :],
                                projw[ki][:, m0:m0 + msz],
                                o_sb[ki][:, 512 * t:512 * (t + 1)],
                                start=(ki == 0), stop=(ki == 1))
                        st = p2.tile([128, 512], f32, tag="fo_st", bufs=3,
                                     name="fo_st")
                        nc.vector.tensor_copy(st[:msz, :], ps[:msz, :])
                        nc.sync.dma_start(
                            out_d.ap()[m0:m0 + msz, 512 * t:512 * (t + 1)],
                            st[:msz, :])

    nc.compile()
    import os
    trace = bool(os.environ.get("KERNEL_TRACE"))
    res = run_bass_kernel_spmd(nc, host_inputs, core_ids=list(range(B)),
                               trace=trace)
    global LAST_EXEC_NS, LAST_RES
    LAST_EXEC_NS = res.exec_time_ns
    LAST_RES = res
    return res


LAST_EXEC_NS = None
LAST_RES = None


def _prep_inputs(sel, inputs):
    """Build per-core in_maps (weights shared, I slab per batch)."""
    I = np.asarray(inputs["I"], dtype=np.float32)
    ex_ws = [np.asarray(inputs[f"ex_w{j}"], dtype=np.float32) for j in range(12)]
    ex_bs = [np.asarray(inputs[f"ex_b{j}"], dtype=np.float32) for j in range(12)]

    shared = {}
    vecs = np.zeros((DIM, 8), dtype=np.float32)
    for i, j in enumerate(sel):
        ks = KSZ[j]
        kk = ks * ks
        w = ex_ws[j]
        vecs[:, i] = ex_bs[j]
        if GROUPS[j] == 1:
            # [out, in, k, k] -> [in, k*k, out]
            shared[f"e{i}_w"] = np.ascontiguousarray(
                w.transpose(1, 2, 3, 0).reshape(DIM, kk * DIM)).astype(BF16)
        elif ks > 1:
            wv = w[:, 0, :, :].reshape(DIM, kk)   # [c, k*k]
            da = np.zeros((128, kk, 128), dtype=np.float32)
            db = np.zeros((64, kk, 64), dtype=np.float32)
            for c in range(128):
                da[c, :, c] = wv[c]
            for c in range(64):
                db[c, :, c] = wv[128 + c]
            shared[f"e{i}_da"] = da.reshape(128, kk * 128).astype(BF16)
            shared[f"e{i}_db"] = db.reshape(64, kk * 64).astype(BF16)
        else:
            scol = 5 if [x for x in sel[:i]
                         if GROUPS[x] == DIM and KSZ[x] == 1] == [] else 7
            vecs[:, scol] = w[:, 0, 0, 0]
    vecs[:, 4] = np.asarray(inputs["ex_out_b"], dtype=np.float32)
    temp = np.asarray(inputs["temperature"], dtype=np.float32).reshape(HEADS)
    vecs[:, 6] = np.repeat(temp, DIM // HEADS)
    shared["vecs"] = vecs

    exw = np.asarray(inputs["ex_out_w"], dtype=np.float32)  # [192, 768, 3, 3]
    wx = np.zeros((DIM, TOPK, 9, DIM), dtype=np.float32)
    for e in range(TOPK):
        blk = exw[:, 192 * e:192 * (e + 1), :, :]   # [out, cin, 3, 3]
        wx[:, e, :, :] = blk.transpose(1, 2, 3, 0).reshape(DIM, 9, DIM)
    shared["wexout"] = wx.reshape(DIM, TOPK * 9 * DIM).astype(BF16)

    kvdw = np.asarray(inputs["kv_dw_w"], dtype=np.float32)[:, 0].reshape(384, 9)
    for m in range(3):
        d = np.zeros((128, 9, 128), dtype=np.float32)
        for c in range(128):
            d[c, :, c] = kvdw[128 * m + c]
        shared[f"dkv{m}"] = d.reshape(128, 9 * 128).astype(BF16)
    qdw = np.asarray(inputs["q_dw_w"], dtype=np.float32)[:, 0].reshape(DIM, 9)
    da = np.zeros((128, 9, 128), dtype=np.float32)
    db = np.zeros((64, 9, 64), dtype=np.float32)
    for c in range(128):
        da[c, :, c] = qdw[c]
    for c in range(64):
        db[c, :, c] = qdw[128 + c]
    shared["dq_a"] = da.reshape(128, 9 * 128).astype(BF16)
    shared["dq_b"] = db.reshape(64, 9 * 64).astype(BF16)

    kvw = np.asarray(inputs["kv_w"], dtype=np.float32)[:, :, 0, 0]  # [384,192]
    shared["kvw"] = np.ascontiguousarray(kvw.T).astype(BF16)
    pw = np.asarray(inputs["proj_w"], dtype=np.float32)[:, :, 0, 0]
    shared["projw"] = np.ascontiguousarray(pw.T).astype(BF16)
    shared["ident"] = np.eye(128, dtype=np.float32).astype(BF16)

    in_maps = []
    for b in range(B):
        m = dict(shared)
        slab = np.zeros((DIM, S, S), dtype=np.float32)
        slab[:, PAD:PAD + H, PAD:PAD + W] = I[b]
        m["xin"] = slab.astype(BF16)
        in_maps.append(m)
    return in_maps


def kernel(**inputs) -> np.ndarray:
    I = np.asarray(inputs["I"], dtype=np.float32)
    T = np.asarray(inputs["T"], dtype=np.float32)
    pw = np.asarray(inputs["ca1_proj_w"], dtype=np.float32)
    sel = _select_experts(I, T, pw)
    in_maps = _prep_inputs(sel, inputs)
    res = _build_and_run(sel, in_maps)
    out = np.stack([np.asarray(res.results[b]["out"], dtype=np.float32)
                    .reshape(DIM, H, W) for b in range(B)])
    return out

